# revision 26
# baseline (speedup 1.0000x reference)
"""Trainium2 Bass kernel for the pairwise concordance-index loss.

reference:
    loss = sum_{i<j, f_i=f_j=1} relu((p_i-p_j)(t_i-t_j)) / 100 / n_pairs

Math:
  Compact to the n1 flagged entries (f=1), pad with zero rows to NPAD.
  M[i,j] = (p_i-p_j)(t_i-t_j) = A^T B, rank 4:
      A = [u, 1, p, t],  B = [1, u, -t, -p],  u = p*t   (bf16)
  sum relu(M) = 0.5*(sum M + sum |M|); sum M over i<j has an O(n) closed
  form done on the host in fp64 over the same bf16 factors; sum |M| is the
  O(n^2) part.

Work split (NB = NPAD/128 row/col blocks, NB=48 for this input):
  Device: for each row-block A, the cyclic column-blocks at offsets
  e = 1..NB/2-1; |M| is symmetric so each unordered block pair is computed
  exactly once.  Host (exact fp64 over the bf16 factors): e=0 (within-block
  pairs) and e=NB/2 block pairs.

Device (8 cores, identical program, data-sharded): core k owns row-blocks
  6k..6k+5 as 3 pairs.  Per pair (blocks bA,bB): 23*128=2944 columns each,
  generated by K=4 bf16 matmuls packed 4-way into disjoint 32-row PE groups
  (tile_position) -> 4-bank PSUM supertiles [128,4,512].  Each supertile is
  consumed by one abs-row-sum job on the DVE (tensor_reduce XY,
  apply_absolute_value) or the ScalarE (activation Abs + accum_out),
  assigned greedily to balance predicted engine time.
"""

import numpy as np

P = 128
NCORE = 8
E0_DEV = 4          # device covers block offsets e in [E0_DEV, NB/2-1];
                    # e in [0, E0_DEV) and e = NB/2 are exact on the host

_cache = {}


def _plan(nb):
    """Static per-core schedule: NB total blocks, nb_core = nb//NCORE (even).

    Device covers offsets e in [E0_DEV, nb/2-1]: (nb/2-E0_DEV)*128 = 2560
    columns per block = exactly 5 chunks of 512.  Local block j's columns
    sit at b_sb offset 128*j..+2560.

    Entries are (local block j, col offset, N, tile_index, bank, group).
    Groups 0,1 hold block A weights (chunks 0-2 / 3-4), groups 2,3 block
    B — each PE row-group only reads a narrow b_cols range.
    Tiles per pair: 0=[A0,A3] 1=[B0,B3] 2=[A1,A4] 3=[B1,B4] 4=[A2,B2].
    """
    nb_core = nb // NCORE
    assert nb_core % 2 == 0
    ncols = (nb // 2 - E0_DEV) * P
    assert ncols == 2560, ncols
    pairs = []
    for pr in range(nb_core // 2):
        jA, jB = 2 * pr, 2 * pr + 1
        A = [(jA, P * jA + 512 * c, 512) for c in range(5)]
        B = [(jB, P * jB + 512 * c, 512) for c in range(5)]
        quads = [
            [A[0] + (0, 0, 0), A[3] + (0, 1, 1),
             B[0] + (1, 0, 2), B[3] + (1, 1, 3)],
            [A[1] + (2, 0, 0), A[4] + (2, 1, 1),
             B[1] + (3, 0, 2), B[4] + (3, 1, 3)],
            [A[2] + (4, 0, 0), B[2] + (4, 1, 2)],
        ]
        pairs.append(quads)
    return pairs


def _group_ranges(nb):
    """Per-PE-row-group b_cols column ranges actually read (see _plan)."""
    nb_core = nb // NCORE
    ncols = (nb // 2 - E0_DEV) * P
    lo_w = 1536                        # chunks 0-2
    ranges = [
        (0, P * (nb_core - 2) + lo_w),             # g0: A chunks 0-2
        (lo_w, P * (nb_core - 2) + ncols),         # g1: A chunks 3-4
        (P, P * (nb_core - 1) + lo_w),             # g2: B chunks 0-2
        (P + lo_w, P * (nb_core - 1) + ncols),     # g3: B chunks 3-4
    ]
    return ranges


def _build(nb):
    """Build + compile the Bass module (once per process)."""
    import concourse.bacc as bacc
    import concourse.tile as tile
    import concourse.mybir as mybir

    f32 = mybir.dt.float32
    bf16 = mybir.dt.bfloat16
    nb_core = nb // NCORE
    ncols = (nb // 2 - E0_DEV) * P
    awidth = P * (nb_core // 2)        # a_rows cols: one 128-col slab per pair
    bwidth = P * (nb_core - 1) + ncols # b_cols width (3200 for nb=48)
    pairs = _plan(nb)
    njobs = sum(5 for _ in pairs) + 1  # one job per 2-bank tile; last split

    nc = bacc.Bacc("TRN2", target_bir_lowering=False, debug=False,
                   num_devices=NCORE)
    a_dram = nc.dram_tensor("a_rows", [P, awidth], bf16, kind="ExternalInput")
    b_dram = nc.dram_tensor("b_cols", [4, bwidth], bf16, kind="ExternalInput")
    acc_dram = nc.dram_tensor("acc", [P, njobs], f32, kind="ExternalOutput")

    # effective job cost (ns) per engine (HW-measured incl. inter-op gap)
    def dve_cost(fd):
        return (40 + fd) / 0.96

    def act_cost(fd):
        return (fd + 315) / 1.2

    with tile.TileContext(nc) as tc:
        with (
            tc.tile_pool(name="inp", bufs=1) as inp_pool,
            tc.tile_pool(name="accp", bufs=1) as acc_pool,
            tc.tile_pool(name="ps", bufs=4, space="PSUM") as ps,
        ):
            a_sb = inp_pool.tile([P, awidth], bf16)
            b_sb = inp_pool.tile([P, bwidth], bf16)
            acc_sb = acc_pool.tile([P, njobs], f32)

            # replicate the B-factor rows into the four 32-row PE groups,
            # each only over the column range that group actually reads.
            # Two HWDGE queues; a_rows first on SP (it gates LDWEIGHTS),
            # groups ordered by first use.
            granges = _group_ranges(nb)
            # only partition rows 32g..32g+3 of a_sb are ever read - two
            # narrow DMAs instead of the full [128, awidth] scatter
            nc.sync.dma_start(a_sb[0:36, :], a_dram.ap()[0:36, :])
            nc.scalar.dma_start(a_sb[64:100, :], a_dram.ap()[64:100, :])
            for q, g in ((nc.scalar, 0), (nc.sync, 1),
                         (nc.scalar, 2), (nc.sync, 3)):
                lo, hi = granges[g]
                q.dma_start(b_sb[32 * g:32 * g + 4, lo:hi],
                            b_dram.ap()[:, lo:hi])

            t_dve = 0.0
            t_act = 0.0
            job = 0

            def reduce_job(ap, fd):
                nonlocal t_dve, t_act, job
                if t_dve + dve_cost(fd) <= t_act + act_cost(fd):
                    t_dve += dve_cost(fd)
                    nc.vector.tensor_reduce(
                        acc_sb[:, job:job + 1], ap,
                        axis=mybir.AxisListType.XY, op=mybir.AluOpType.add,
                        apply_absolute_value=True,
                    )
                else:
                    t_act += act_cost(fd)
                    nc.scalar.activation(
                        ap, ap, mybir.ActivationFunctionType.Abs,
                        accum_out=acc_sb[:, job:job + 1],
                    )
                job += 1

            flush = 5 * (len(pairs) - 1)   # acc cols final before last pair
            for pr, quads in enumerate(pairs):
                last_pr = pr == len(pairs) - 1
                tiles = {}
                for quad in quads:
                    for ti in sorted({e[3] for e in quad}):
                        tiles[ti] = ps.tile([P, 2, 512], f32, tag="st",
                                            name=f"st{pr}_{ti}")
                    for (j, off, n, ti, bank, g) in quad:
                        nc.tensor.matmul(
                            tiles[ti][:, bank, 0:n],
                            a_sb[32 * g:32 * g + 4, P * pr:P * pr + P],
                            b_sb[32 * g:32 * g + 4, off:off + n],
                            start=True, stop=True,
                            tile_position=(32 * g, 0),
                        )
                    for ti in sorted({e[3] for e in quad}):
                        if last_pr and ti == 4:
                            # split the final job across both engines
                            reduce_job(tiles[ti][:, 0:1, :], 512)
                            reduce_job(tiles[ti][:, 1:2, :], 512)
                        else:
                            reduce_job(tiles[ti][:, :, :], 1024)
                if last_pr:
                    # flush finished acc columns while the last pair runs
                    # (sync queue is idle mid-kernel; scalar queue is not)
                    nc.sync.dma_start(acc_dram.ap()[:, 0:flush],
                                      acc_sb[:, 0:flush])

            assert job == njobs, job
            nc.sync.dma_start(acc_dram.ap()[:, flush:njobs],
                              acc_sb[:, flush:njobs])

    nc.compile()
    return nc


def _get_nc(nb):
    key = ("nc", nb)
    if key not in _cache:
        _cache[key] = _build(nb)
    return _cache[key]


def _factors(p, t):
    """bf16 rank-4 factors for compacted (all-flagged) entries p, t."""
    import ml_dtypes

    u = (p * t).astype(np.float32)
    one = np.ones_like(p)
    A = np.stack([u, one, p, t]).astype(ml_dtypes.bfloat16)
    B = np.stack([one, u, -t, -p]).astype(ml_dtypes.bfloat16)
    return A, B


def _make_in_maps(A, Bm, nb):
    """Per-core DRAM images from padded bf16 factor matrices [4, NPAD]."""
    npad = nb * P
    nb_core = nb // NCORE
    ncols = (nb // 2 - E0_DEV) * P
    awidth = P * (nb_core // 2)
    bwidth = P * (nb_core - 1) + ncols

    in_maps = []
    for k in range(NCORE):
        a_rows = np.zeros((P, awidth), dtype=A.dtype)
        for pr in range(nb_core // 2):
            bA = nb_core * k + 2 * pr
            bB = bA + 1
            for g in range(4):
                blk = bA if g < 2 else bB
                a_rows[32 * g:32 * g + 4, P * pr:P * pr + P] = \
                    A[:, P * blk:P * blk + P]
        cols = (P * (nb_core * k + E0_DEV) + np.arange(bwidth)) % npad
        b_cols = np.ascontiguousarray(Bm[:, cols])
        in_maps.append({"a_rows": a_rows, "b_cols": b_cols})
    return in_maps


def kernel(pred, gt, gt_fracTime, gt_ifMOF):
    from concourse import bass_utils

    pred = np.asarray(pred)
    gt = np.asarray(gt)
    ift = int(np.asarray(gt_fracTime))
    imf = int(np.asarray(gt_ifMOF))

    p_full = pred.astype(np.float32)
    t_full = gt[:, ift].astype(np.float32)
    f_full = gt[:, imf] == 1

    idx = np.nonzero(f_full)[0]
    n1 = len(idx)
    p = p_full[idx]
    t = t_full[idx]

    # pad so blocks split into 8 cores * (even block count)
    gran = NCORE * 2 * P
    npad = max(gran * 2, ((n1 + gran - 1) // gran) * gran)
    nb = npad // P

    A, Bm = _factors(p, t)
    Ap = np.zeros((4, npad), dtype=A.dtype)
    Bp = np.zeros((4, npad), dtype=Bm.dtype)
    Ap[:, :n1] = A
    Bp[:, :n1] = Bm

    in_maps = _make_in_maps(Ap, Bp, nb)
    nc = _get_nc(nb)
    res = bass_utils.run_bass_kernel_spmd(nc, in_maps,
                                          core_ids=list(range(NCORE)))

    # device part: sum |M| over block pairs at cyclic offsets E0_DEV..NB/2-1
    T = 0.0
    for r in res.results:
        T += r["acc"].astype(np.float64).sum()

    # host parts in fp64 over the same bf16 values the device used
    A64 = Ap.astype(np.float64).reshape(4, nb, P)
    B64 = Bp.astype(np.float64).reshape(4, nb, P)
    # e=0: within-block pairs i<j  (diagonal i=j excluded exactly)
    Md = np.einsum('kba,kbc->bac', A64, B64, optimize=True)
    for b in range(nb):
        np.fill_diagonal(Md[b], 0.0)
    T += 0.5 * np.abs(Md).sum()
    # e = 1..E0_DEV-1 block pairs (A, A+e), each unordered pair once
    for e in range(1, E0_DEV):
        Me = np.einsum('kba,kbc->bac', A64, np.roll(B64, -e, axis=1),
                       optimize=True)
        T += np.abs(Me).sum()
    # e=NB/2 block pairs, each unordered pair once
    h = nb // 2
    Me = np.einsum('kba,kbc->bac', A64[:, :h], B64[:, h:], optimize=True)
    T += np.abs(Me).sum()

    # signed closed form: sum_{i<j} M = (sum_{ij} M - sum_ii M)/2
    Af = Ap.astype(np.float64)
    Bf = Bp.astype(np.float64)
    S_all = (Af.sum(axis=1) * Bf.sum(axis=1)).sum()
    D_diag = (Af * Bf).sum()
    S_half = (S_all - D_diag) / 2.0

    n1f = float(n1)
    n_pairs = (n1f * n1f - n1f) / 2.0
    loss = 0.5 * (S_half + T) / 100.0 / n_pairs
    return np.asarray(np.float32(loss))


# revision 29
# speedup vs baseline: 1.0642x; 1.0642x over previous
"""Trainium2 Bass kernel for the pairwise concordance-index loss.

reference:
    loss = sum_{i<j, f_i=f_j=1} relu((p_i-p_j)(t_i-t_j)) / 100 / n_pairs

Math:
  Compact to the n1 flagged entries (f=1), pad with zero rows to NPAD.
  M[i,j] = (p_i-p_j)(t_i-t_j) = A^T B, rank 4:
      A = [u, 1, p, t],  B = [1, u, -t, -p],  u = p*t   (bf16)
  sum relu(M) = 0.5*(sum M + sum |M|); sum M over i<j has an O(n) closed
  form done on the host in fp64 over the same bf16 factors; sum |M| is the
  O(n^2) part.

Work split (NB = NPAD/128 row/col blocks, NB=48 for this input):
  Device: for each row-block A, the cyclic column-blocks at offsets
  e in [E0_DEV, NB/2-1]; |M| is symmetric so each unordered block pair is
  computed exactly once.  Host (exact fp64 over the same bf16 factors):
  offsets e in [0, E0_DEV) (incl. the within-block e=0 pairs) and e=NB/2.

Device (8 cores, identical program, data-sharded): core k owns row-blocks
  6k..6k+5 as 3 pairs.  Per pair (blocks bA,bB): 20*128=2560 columns each
  (5 chunks of 512), generated by K=4 bf16 matmuls packed 4-way into
  disjoint 32-row PE groups (tile_position) -> 2-bank PSUM tiles
  [128,2,512] (4 in flight).  Each tile is consumed by one FD=1024
  abs-row-sum job on the DVE (tensor_reduce XY, apply_absolute_value) or
  the ScalarE (activation Abs + accum_out), assigned greedily from
  HW-measured per-engine costs; both engines run ~100% busy in steady
  state, which is the transit roofline (every PSUM element must pass
  through DVE or ScalarE exactly once; GPSIMD has no PSUM port).
"""

import numpy as np

P = 128
NCORE = 8
E0_DEV = 4          # device covers block offsets e in [E0_DEV, NB/2-1];
                    # e in [0, E0_DEV) and e = NB/2 are exact on the host

_cache = {}


def _plan(nb):
    """Static per-core schedule: NB total blocks, nb_core = nb//NCORE (even).

    Device covers offsets e in [E0_DEV, nb/2-1]: (nb/2-E0_DEV)*128 = 2560
    columns per block = exactly 5 chunks of 512.  Local block j's columns
    sit at b_sb offset 128*j..+2560.

    Entries are (local block j, col offset, N, tile_index, bank, group).
    Groups 0,1 hold block A weights (chunks 0-2 / 3-4), groups 2,3 block
    B — each PE row-group only reads a narrow b_cols range.
    Tiles per pair: 0=[A0,A3] 1=[B0,B3] 2=[A1,A4] 3=[B1,B4] 4=[A2,B2].
    """
    nb_core = nb // NCORE
    assert nb_core % 2 == 0
    ncols = (nb // 2 - E0_DEV) * P
    assert ncols == 2560, ncols
    pairs = []
    for pr in range(nb_core // 2):
        jA, jB = 2 * pr, 2 * pr + 1
        A = [(jA, P * jA + 512 * c, 512) for c in range(5)]
        B = [(jB, P * jB + 512 * c, 512) for c in range(5)]
        quads = [
            [A[0] + (0, 0, 0), A[3] + (0, 1, 1),
             B[0] + (1, 0, 2), B[3] + (1, 1, 3)],
            [A[1] + (2, 0, 0), A[4] + (2, 1, 1),
             B[1] + (3, 0, 2), B[4] + (3, 1, 3)],
            [A[2] + (4, 0, 0), B[2] + (4, 1, 2)],
        ]
        pairs.append(quads)
    return pairs


def _group_ranges(nb):
    """Per-PE-row-group b_cols column ranges actually read (see _plan)."""
    nb_core = nb // NCORE
    ncols = (nb // 2 - E0_DEV) * P
    lo_w = 1536                        # chunks 0-2
    ranges = [
        (0, P * (nb_core - 2) + lo_w),             # g0: A chunks 0-2
        (lo_w, P * (nb_core - 2) + ncols),         # g1: A chunks 3-4
        (P, P * (nb_core - 1) + lo_w),             # g2: B chunks 0-2
        (P + lo_w, P * (nb_core - 1) + ncols),     # g3: B chunks 3-4
    ]
    return ranges


def _build(nb):
    """Build + compile the Bass module (once per process)."""
    import concourse.bacc as bacc
    import concourse.tile as tile
    import concourse.mybir as mybir

    f32 = mybir.dt.float32
    bf16 = mybir.dt.bfloat16
    nb_core = nb // NCORE
    ncols = (nb // 2 - E0_DEV) * P
    awidth = P * (nb_core // 2)        # a_rows cols: one 128-col slab per pair
    bwidth = P * (nb_core - 1) + ncols # b_cols width (3200 for nb=48)
    pairs = _plan(nb)
    njobs = sum(5 for _ in pairs) + 1  # one job per 2-bank tile; last split

    nc = bacc.Bacc("TRN2", target_bir_lowering=False, debug=False,
                   num_devices=NCORE)
    a_dram = nc.dram_tensor("a_rows", [P, awidth], bf16, kind="ExternalInput")
    b_dram = nc.dram_tensor("b_cols", [4, bwidth], bf16, kind="ExternalInput")
    acc_dram = nc.dram_tensor("acc", [P, njobs], f32, kind="ExternalOutput")

    # effective job cost (ns) per engine (HW-measured incl. inter-op gap)
    def dve_cost(fd):
        return (40 + fd) / 0.96

    def act_cost(fd):
        return (fd + 315) / 1.2

    with tile.TileContext(nc) as tc:
        with (
            tc.tile_pool(name="inp", bufs=1) as inp_pool,
            tc.tile_pool(name="accp", bufs=1) as acc_pool,
            tc.tile_pool(name="ps", bufs=4, space="PSUM") as ps,
        ):
            a_sb = inp_pool.tile([P, awidth], bf16)
            b_sb = inp_pool.tile([P, bwidth], bf16)
            acc_sb = acc_pool.tile([P, njobs], f32)

            # replicate the B-factor rows into the four 32-row PE groups,
            # each only over the column range that group actually reads.
            # a_rows first on SP (it gates all LDWEIGHTS); b groups spread
            # across the two HWDGE queues in first-use order.
            granges = _group_ranges(nb)
            nc.sync.dma_start(a_sb[:, :], a_dram.ap()[:, :])
            for q, g in ((nc.scalar, 0), (nc.sync, 1),
                         (nc.scalar, 2), (nc.sync, 3)):
                lo, hi = granges[g]
                q.dma_start(b_sb[32 * g:32 * g + 4, lo:hi],
                            b_dram.ap()[:, lo:hi])

            t_dve = 0.0
            t_act = 0.0
            job = 0

            def reduce_job(ap, fd):
                nonlocal t_dve, t_act, job
                if t_dve + dve_cost(fd) <= t_act + act_cost(fd):
                    t_dve += dve_cost(fd)
                    nc.vector.tensor_reduce(
                        acc_sb[:, job:job + 1], ap,
                        axis=mybir.AxisListType.XY, op=mybir.AluOpType.add,
                        apply_absolute_value=True,
                    )
                else:
                    t_act += act_cost(fd)
                    nc.scalar.activation(
                        ap, ap, mybir.ActivationFunctionType.Abs,
                        accum_out=acc_sb[:, job:job + 1],
                    )
                job += 1

            flush = 5 * (len(pairs) - 1)   # acc cols final before last pair
            for pr, quads in enumerate(pairs):
                last_pr = pr == len(pairs) - 1
                tiles = {}
                for quad in quads:
                    for ti in sorted({e[3] for e in quad}):
                        tiles[ti] = ps.tile([P, 2, 512], f32, tag="st",
                                            name=f"st{pr}_{ti}")
                    for (j, off, n, ti, bank, g) in quad:
                        nc.tensor.matmul(
                            tiles[ti][:, bank, 0:n],
                            a_sb[32 * g:32 * g + 4, P * pr:P * pr + P],
                            b_sb[32 * g:32 * g + 4, off:off + n],
                            start=True, stop=True,
                            tile_position=(32 * g, 0),
                        )
                    for ti in sorted({e[3] for e in quad}):
                        if last_pr and ti == 4:
                            # split the final job across both engines
                            reduce_job(tiles[ti][:, 0:1, :], 512)
                            reduce_job(tiles[ti][:, 1:2, :], 512)
                        else:
                            reduce_job(tiles[ti][:, :, :], 1024)
                if last_pr:
                    # flush finished acc columns while the last pair runs
                    # (sync queue is idle mid-kernel; scalar queue is not)
                    nc.sync.dma_start(acc_dram.ap()[:, 0:flush],
                                      acc_sb[:, 0:flush])

            assert job == njobs, job
            nc.sync.dma_start(acc_dram.ap()[:, flush:njobs],
                              acc_sb[:, flush:njobs])

    nc.compile()
    return nc


def _get_nc(nb):
    key = ("nc", nb)
    if key not in _cache:
        _cache[key] = _build(nb)
    return _cache[key]


def _factors(p, t):
    """bf16 rank-4 factors for compacted (all-flagged) entries p, t."""
    import ml_dtypes

    u = (p * t).astype(np.float32)
    one = np.ones_like(p)
    A = np.stack([u, one, p, t]).astype(ml_dtypes.bfloat16)
    B = np.stack([one, u, -t, -p]).astype(ml_dtypes.bfloat16)
    return A, B


def _make_in_maps(A, Bm, nb):
    """Per-core DRAM images from padded bf16 factor matrices [4, NPAD]."""
    npad = nb * P
    nb_core = nb // NCORE
    ncols = (nb // 2 - E0_DEV) * P
    awidth = P * (nb_core // 2)
    bwidth = P * (nb_core - 1) + ncols

    in_maps = []
    for k in range(NCORE):
        a_rows = np.zeros((P, awidth), dtype=A.dtype)
        for pr in range(nb_core // 2):
            bA = nb_core * k + 2 * pr
            bB = bA + 1
            for g in range(4):
                blk = bA if g < 2 else bB
                a_rows[32 * g:32 * g + 4, P * pr:P * pr + P] = \
                    A[:, P * blk:P * blk + P]
        cols = (P * (nb_core * k + E0_DEV) + np.arange(bwidth)) % npad
        b_cols = np.ascontiguousarray(Bm[:, cols])
        in_maps.append({"a_rows": a_rows, "b_cols": b_cols})
    return in_maps


def kernel(pred, gt, gt_fracTime, gt_ifMOF):
    from concourse import bass_utils

    pred = np.asarray(pred)
    gt = np.asarray(gt)
    ift = int(np.asarray(gt_fracTime))
    imf = int(np.asarray(gt_ifMOF))

    p_full = pred.astype(np.float32)
    t_full = gt[:, ift].astype(np.float32)
    f_full = gt[:, imf] == 1

    idx = np.nonzero(f_full)[0]
    n1 = len(idx)
    p = p_full[idx]
    t = t_full[idx]

    # pad so blocks split into 8 cores * (even block count)
    gran = NCORE * 2 * P
    npad = max(gran * 2, ((n1 + gran - 1) // gran) * gran)
    nb = npad // P

    A, Bm = _factors(p, t)
    Ap = np.zeros((4, npad), dtype=A.dtype)
    Bp = np.zeros((4, npad), dtype=Bm.dtype)
    Ap[:, :n1] = A
    Bp[:, :n1] = Bm

    in_maps = _make_in_maps(Ap, Bp, nb)
    nc = _get_nc(nb)
    res = bass_utils.run_bass_kernel_spmd(nc, in_maps,
                                          core_ids=list(range(NCORE)))

    # device part: sum |M| over block pairs at cyclic offsets E0_DEV..NB/2-1
    T = 0.0
    for r in res.results:
        T += r["acc"].astype(np.float64).sum()

    # host parts in fp64 over the same bf16 values the device used
    A64 = Ap.astype(np.float64).reshape(4, nb, P)
    B64 = Bp.astype(np.float64).reshape(4, nb, P)
    # e=0: within-block pairs i<j  (diagonal i=j excluded exactly)
    Md = np.einsum('kba,kbc->bac', A64, B64, optimize=True)
    for b in range(nb):
        np.fill_diagonal(Md[b], 0.0)
    T += 0.5 * np.abs(Md).sum()
    # e = 1..E0_DEV-1 block pairs (A, A+e), each unordered pair once
    for e in range(1, E0_DEV):
        Me = np.einsum('kba,kbc->bac', A64, np.roll(B64, -e, axis=1),
                       optimize=True)
        T += np.abs(Me).sum()
    # e=NB/2 block pairs, each unordered pair once
    h = nb // 2
    Me = np.einsum('kba,kbc->bac', A64[:, :h], B64[:, h:], optimize=True)
    T += np.abs(Me).sum()

    # signed closed form: sum_{i<j} M = (sum_{ij} M - sum_ii M)/2
    Af = Ap.astype(np.float64)
    Bf = Bp.astype(np.float64)
    S_all = (Af.sum(axis=1) * Bf.sum(axis=1)).sum()
    D_diag = (Af * Bf).sum()
    S_half = (S_all - D_diag) / 2.0

    n1f = float(n1)
    n_pairs = (n1f * n1f - n1f) / 2.0
    loss = 0.5 * (S_half + T) / 100.0 / n_pairs
    return np.asarray(np.float32(loss))
